# revision 11
# baseline (speedup 1.0000x reference)
"""GATv2 attention layer (B=2, T=1024, C_IN=128, D=64) on 8 trn2 NeuronCores.

Sharding: flatten (B, T) destination rows -> 2048 rows, 256 per core.
Each core gets fp16 host-prepared layouts: feat^T of its batch, its own 256
rows' feat^T slice (for k), feat in 128-row blocks (final matmul rhs),
[W1^T | W2^T], the score weight A32s, plus its fp16 adj rows.

Per-core algorithm (i = destination row, j = source node, d = head dim 64):
  scores[i, j] = sum_d a[d] * relu(q[j, d] + k[i, d])
Layout trick: qT2 = [q^T; q^T] stacked [128(=2x64 d), 1024(=j)] in fp16.
For a PAIR of rows (2p, 2p+1), bias column kpair[:, p] = [k[2p]; k[2p+1]]:
  E2 = relu(qT2 + kpair[:, p])   one DVE tensor_scalar / ACT activation
  (DVE at 4 fp16/cyc/lane takes 48 of the 64 pair tiles, ACT 1/cyc takes 16;
  this 3:1 split matches their measured rates, both engines saturate)
  scores come from a PE matmul with lhsT = A32s slot q=p%16, an [128, 32]
  fp16 matrix holding `a` in column 2q (top d-half) and 2q+1 (bottom d-half),
  zeros elsewhere. 16 pairs accumulate into one 32-row psum band, so the
  matmul psum base stays 32-aligned (hardware requirement) while every
  logical row ends up at psum partition 2p+{0,1}. Consecutive matmuls are
  issued to different PSUM col-groups so they overlap on the PE sub-arrays.
Mask fold: softmax is shift-invariant, so exp(s)*adj == exp(s + 30*adj - 30)
(e^-30 ~ 1e-13 kills masked entries; unmasked are exact). The +30*adj term
is accumulated INTO the score psum by one extra small matmul per 32-row band
(lhsT = a 30-scaled identity block, rhs = this band's adj rows), and the -30
rides the exp activation's free bias — the mask costs the Vector engine
nothing.
Final: out[i, :] = (att_unnorm @ feat) / rowsum(att_unnorm); att transposed
on PE, rowsum from a ones-column appended to the feat blocks; the reciprocal
scale rides the ACT copy's free per-partition scale operand.
Both i-tiles' pair loops are emitted before either softmax phase so the
scheduler overlaps i-tile 0's softmax with i-tile 1's pair loop.
"""
import sys

sys.path.insert(0, "/opt/trn_rl_repo")

from contextlib import ExitStack

import numpy as np

import concourse.bass as bass  # noqa: F401
import concourse.tile as tile
from concourse import bacc, masks, mybir
from concourse.bass_utils import run_bass_kernel_spmd

B, T, C_IN, D = 2, 1024, 128, 64
N_CORES = 8
ROWS = (B * T) // N_CORES  # 256 destination rows per core
CPB = N_CORES // B  # cores per batch
NT = T // 128  # token tiles
NIT = ROWS // 128  # i-tiles per core
NPAIR = 64  # row pairs per i-tile
NSLOT = 16  # pair slots per 32-row psum band

FP32 = mybir.dt.float32
FP16 = mybir.dt.float16
AX = mybir.AxisListType.X
OP = mybir.AluOpType
AF = mybir.ActivationFunctionType

MASK_BIAS = 30.0  # exp(s + 30*adj - 30): e^-30 zeroes masked entries in fp16


def _emit(ctx, tc, nc, featT16, featkT16, feat16b, wT16_in, adj, a32, out):
    singles = ctx.enter_context(tc.tile_pool(name="singles", bufs=1))
    ident16 = singles.tile([128, 128], FP16)
    masks.make_identity(nc, ident16[:])
    ident30 = singles.tile([128, 128], FP16)
    nc.vector.tensor_scalar(ident30[:], ident16[:], MASK_BIAS, None, OP.mult)
    nbias = singles.tile([128, 1], FP32)
    nc.vector.memset(nbias[:], -MASK_BIAS)
    feat16 = singles.tile([128, NT * (C_IN + 1)], FP16)  # feat blocks + ones col
    qT2 = singles.tile([128, T], FP16)
    kpair = singles.tile([128, ROWS // 2], FP32)
    A32s = singles.tile([128, NSLOT * 32], FP16)
    wT16 = singles.tile([128, 2 * D], FP16)
    adj_sb = singles.tile([128, NIT * T], FP16)  # both i-tiles' adj rows

    with ExitStack() as sctx:
        spsum = sctx.enter_context(tc.tile_pool(name="setup_ps", bufs=4, space="PSUM"))
        spool = sctx.enter_context(tc.tile_pool(name="setup_sb", bufs=1))

        # Spread input DMAs over the three issuing queues (sync/scalar/gpsimd
        # use separate DMA rings, and same-queue transfers serialize); each
        # DMA pays ~2.3us fixed latency (issue+DGE+sem-prop), so the three
        # tensors on the qT2/kpair critical path each lead their queue.
        fT = spool.tile([128, T], FP16, tag="fT")
        nc.gpsimd.dma_start(fT[:], featT16[:, :])
        nc.sync.dma_start(wT16[:], wT16_in[:, :])
        fkT = spool.tile([128, ROWS], FP16, tag="fkT")
        nc.scalar.dma_start(fkT[:], featkT16[:, :])
        nc.sync.dma_start(A32s[:], a32[:, :])
        for it in range(NIT):
            nc.gpsimd.dma_start(
                adj_sb[:, it * T : (it + 1) * T], adj[it * 128 : (it + 1) * 128, :]
            )
        nc.scalar.dma_start(feat16[:], feat16b[:, :])

        # kT = W2^T.T @ featkT  [64, ROWS] -> kpair columns [k(2p); k(2p+1)]
        kps = spsum.tile([64, ROWS], FP32, tag="qk")
        nc.tensor.matmul(kps[:], wT16[:, D : 2 * D], fkT[:], start=True, stop=True)
        kpv = kps[:].rearrange("d (p two) -> d two p", two=2)
        nc.vector.tensor_copy(kpair[0:64, :], kpv[:, 0, :])
        nc.vector.tensor_copy(kpair[64:128, :], kpv[:, 1, :])

        # qT = W1^T.T @ featT   [64, T] -> stacked fp16 qT2
        for h in range(T // 512):
            ps = spsum.tile([64, 512], FP32, tag="qk")
            nc.tensor.matmul(
                ps[:], wT16[:, 0:D], fT[:, h * 512 : (h + 1) * 512], start=True, stop=True
            )
            nc.vector.tensor_copy(qT2[0:64, h * 512 : (h + 1) * 512], ps[:])
            nc.scalar.copy(qT2[64:128, h * 512 : (h + 1) * 512], ps[:])

    # separate pools per producer so DVE buffer recycling never waits on the
    # slower ACT tiles' matmuls (idx%4==3 goes to ACT)
    e2pool = ctx.enter_context(tc.tile_pool(name="e2", bufs=5))
    e2vpool = ctx.enter_context(tc.tile_pool(name="e2v", bufs=4))
    e2apool = ctx.enter_context(tc.tile_pool(name="e2a", bufs=4))
    softpool = ctx.enter_context(tc.tile_pool(name="soft", bufs=2))
    smallpool = ctx.enter_context(tc.tile_pool(name="small", bufs=2))
    attTpool = ctx.enter_context(tc.tile_pool(name="attT", bufs=2))
    outpool = ctx.enter_context(tc.tile_pool(name="outp", bufs=2))
    ps_scores = ctx.enter_context(tc.tile_pool(name="ps_s", bufs=4, space="PSUM"))
    ps_tr = ctx.enter_context(tc.tile_pool(name="ps_tr", bufs=2, space="PSUM"))
    ps_out = ctx.enter_context(tc.tile_pool(name="ps_o", bufs=2, space="PSUM"))

    # --- pair loops for both i-tiles, emitted before any softmax work ---
    sc = []
    for it in range(NIT):
        s0 = ps_scores.tile([128, 512], FP32, tag="s")
        s1 = ps_scores.tile([128, 512], FP32, tag="s")
        sc.append((s0, s1))
        # visit pairs q-major so consecutive matmuls hit different PSUM
        # col-groups (tile_position col 32g) and overlap on the PE sub-arrays
        e2big = None
        for idx in range(NPAIR):
            q, g = divmod(idx, 4)
            p = NSLOT * g + q
            P = it * NPAIR + p
            r = idx % 4
            if r == 0:
                e2big = e2pool.tile([128, 2 * T], FP16, tag="e2")
                e2 = e2big[:, 0:T]
            elif r == 1:
                e2 = e2big[:, T : 2 * T]
            elif r == 2:
                e2v = e2vpool.tile([128, T], FP16, tag="e2v")
                e2 = e2v[:]
            else:
                e2a = e2apool.tile([128, T], FP16, tag="e2a")
                e2 = e2a[:]
            kcol = kpair[:, P : P + 1]
            if r == 3:
                nc.scalar.activation(e2[:], qT2[:], AF.Relu, bias=kcol)
            else:
                nc.vector.tensor_scalar(e2[:], qT2[:], kcol, 0.0, OP.add, OP.max)
            lhsT = A32s[:, 32 * q : 32 * q + 32]
            first = q == 0
            nc.tensor.matmul(
                s0[32 * g : 32 * g + 32, :],
                lhsT,
                e2[:, 0:512],
                start=first,
                stop=False,
                tile_position=(0, 32 * g),
                skip_group_check=True,
            )
            nc.tensor.matmul(
                s1[32 * g : 32 * g + 32, :],
                lhsT,
                e2[:, 512:T],
                start=first,
                stop=False,
                tile_position=(0, 32 * g),
                skip_group_check=True,
            )
            if q == NSLOT - 1:
                # close this band's accumulation with the mask term:
                # s[band g] += 30 * adj[band g rows]  (lhsT = 30*I32 block)
                i30 = ident30[32 * g : 32 * g + 32, 32 * g : 32 * g + 32]
                arows = adj_sb[32 * g : 32 * g + 32, it * T : (it + 1) * T]
                for hh, sps in enumerate((s0, s1)):
                    nc.tensor.matmul(
                        sps[32 * g : 32 * g + 32, :],
                        i30,
                        arows[:, hh * 512 : (hh + 1) * 512],
                        start=False,
                        stop=True,
                        tile_position=(32 * g, 32 * g),
                        skip_group_check=True,
                    )

    # --- softmax + output, per i-tile (scheduler overlaps with pair loops) ---
    for it in range(NIT):
        s0, s1 = sc[it]
        pexp = softpool.tile([128, T], FP16, tag="pexp")
        pst = ps_tr.tile([128, T], FP16, tag="tr")
        attT = attTpool.tile([128, T], FP16, tag="attT")
        for hh in range(2):
            lo = hh * 512
            nc.scalar.activation(
                pexp[:, lo : lo + 512], (s0, s1)[hh][:], AF.Exp, bias=nbias[:]
            )
            for t in range(lo // 128, (lo + 512) // 128):
                nc.tensor.transpose(
                    pst[:, t * 128 : (t + 1) * 128], pexp[:, t * 128 : (t + 1) * 128], ident16[:]
                )
            # PSUM->SBUF copies split across the two capable engines
            if hh == 0:
                nc.scalar.copy(attT[:, lo : lo + 512], pst[:, lo : lo + 512])
            else:
                nc.vector.tensor_copy(attT[:, lo : lo + 512], pst[:, lo : lo + 512])

        W = C_IN + 1
        po = ps_out.tile([128, W], FP32, tag="o")
        for t in range(NT):
            nc.tensor.matmul(
                po[:],
                attT[:, t * 128 : (t + 1) * 128],
                feat16[:, t * W : (t + 1) * W],
                start=(t == 0),
                stop=(t == NT - 1),
            )
        inv = smallpool.tile([128, 1], FP32, tag="inv")
        nc.vector.reciprocal(inv[:], po[:, C_IN : C_IN + 1])
        out_sb = outpool.tile([128, C_IN], FP32, tag="out")
        nc.scalar.activation(out_sb[:], po[:, 0:C_IN], AF.Copy, bias=0.0, scale=inv[:])
        nc.sync.dma_start(out[it * 128 : (it + 1) * 128, :], out_sb[:])


_PROGRAM = None


def build_program():
    global _PROGRAM
    if _PROGRAM is not None:
        return _PROGRAM
    nc = bacc.Bacc("TRN2", target_bir_lowering=False, debug=False, num_devices=N_CORES)
    featT16 = nc.dram_tensor("featT16", [C_IN, T], FP16, kind="ExternalInput")
    featkT16 = nc.dram_tensor("featkT16", [C_IN, ROWS], FP16, kind="ExternalInput")
    feat16b = nc.dram_tensor("feat16b", [128, NT * (C_IN + 1)], FP16, kind="ExternalInput")
    wT16_in = nc.dram_tensor("wT16", [C_IN, 2 * D], FP16, kind="ExternalInput")
    adj = nc.dram_tensor("adj", [ROWS, T], FP16, kind="ExternalInput")
    a32 = nc.dram_tensor("a32", [128, NSLOT * 32], FP16, kind="ExternalInput")
    out = nc.dram_tensor("out", [ROWS, C_IN], FP32, kind="ExternalOutput")
    with tile.TileContext(nc) as tc:
        with ExitStack() as ctx:
            _emit(ctx, tc, nc, featT16, featkT16, feat16b, wT16_in, adj, a32, out)
    nc.compile()
    _PROGRAM = nc
    return nc


def make_a32(a):
    a32 = np.zeros((128, NSLOT * 32), dtype=np.float16)
    for q in range(NSLOT):
        a32[0:64, 32 * q + 2 * q] = a
        a32[64:128, 32 * q + 2 * q + 1] = a
    return a32


def make_in_maps(feat, adj, W1, W2, a):
    feat = np.ascontiguousarray(feat, dtype=np.float32)
    adj = np.ascontiguousarray(adj, dtype=np.float32)
    W1 = np.asarray(W1, dtype=np.float32)
    W2 = np.asarray(W2, dtype=np.float32)
    a32 = make_a32(np.asarray(a, dtype=np.float32))
    wT16 = np.ascontiguousarray(
        np.concatenate([W1.T, W2.T], axis=1).astype(np.float16)
    )  # [128, 128]
    in_maps = []
    for b in range(B):
        feat16 = feat[b].astype(np.float16)  # [T, C_IN]
        fT = np.ascontiguousarray(feat16.T)  # [C_IN, T]
        fb = feat16.reshape(NT, 128, C_IN).transpose(1, 0, 2)  # [128, NT, C_IN]
        fblk = np.concatenate(
            [fb, np.ones((128, NT, 1), dtype=np.float16)], axis=2
        ).reshape(128, NT * (C_IN + 1))
        fblk = np.ascontiguousarray(fblk)
        for cc in range(CPB):
            r0 = cc * ROWS
            in_maps.append(
                {
                    "featT16": fT,
                    "featkT16": np.ascontiguousarray(fT[:, r0 : r0 + ROWS]),
                    "feat16b": fblk,
                    "wT16": wT16,
                    "adj": np.ascontiguousarray(adj[b, r0 : r0 + ROWS].astype(np.float16)),
                    "a32": a32,
                }
            )
    return in_maps


def run(feat, adj, W1, W2, a, trace=False):
    nc = build_program()
    in_maps = make_in_maps(feat, adj, W1, W2, a)
    last_err = None
    for attempt in range(3):
        try:
            res = run_bass_kernel_spmd(
                nc, in_maps, core_ids=list(range(N_CORES)), trace=trace
            )
            outs = [np.asarray(res.results[c]["out"]) for c in range(N_CORES)]
            break
        except Exception as e:  # transient NRT device errors recover on retry
            last_err = e
            import time

            time.sleep(5)
    else:
        raise last_err
    full = np.concatenate(outs, axis=0).reshape(B, T, C_IN).astype(np.float32)
    return full, res


def kernel(feat, adj, W1, W2, a):
    full, _ = run(feat, adj, W1, W2, a)
    return full


# revision 15
# speedup vs baseline: 1.0719x; 1.0719x over previous
"""GATv2 attention layer (B=2, T=1024, C_IN=128, D=64) on 8 trn2 NeuronCores.

Sharding: flatten (B, T) destination rows -> 2048 rows, 256 per core.
Each core gets fp16 host-prepared layouts: feat^T of its batch, its own 256
rows' feat^T slice (for k), feat in 128-row blocks (final matmul rhs),
[W1^T | W2^T], the score weight A32s, plus its fp16 adj rows.

Per-core algorithm (i = destination row, j = source node, d = head dim 64):
  scores[i, j] = sum_d a[d] * relu(q[j, d] + k[i, d])
Layout trick: qT2 = [q^T; q^T] stacked [128(=2x64 d), 1024(=j)] in fp16.
For a PAIR of rows (2p, 2p+1), bias column kpair[:, p] = [k[2p]; k[2p+1]]:
  E2 = relu(qT2 + kpair[:, p])   one DVE tensor_scalar / ACT activation
  (DVE at 4 fp16/cyc/lane takes 48 of the 64 pair tiles, ACT 1/cyc takes 16;
  this 3:1 split matches their measured rates, both engines saturate)
  scores come from a PE matmul with lhsT = A32s slot q=p%16, an [128, 32]
  fp16 matrix holding `a` in column 2q (top d-half) and 2q+1 (bottom d-half),
  zeros elsewhere. 16 pairs accumulate into one 32-row psum band, so the
  matmul psum base stays 32-aligned (hardware requirement) while every
  logical row ends up at psum partition 2p+{0,1}. Consecutive matmuls are
  issued to different PSUM col-groups so they overlap on the PE sub-arrays.
Mask fold: softmax is shift-invariant, so exp(s)*adj == exp(s + 30*adj - 30)
(e^-30 ~ 1e-13 kills masked entries; unmasked are exact). The +30*adj term
is accumulated INTO the score psum by one extra small matmul per 32-row band
(lhsT = a 30-scaled identity block, rhs = this band's adj rows), and the -30
rides the exp activation's free bias — the mask costs the Vector engine
nothing.
Final: out[i, :] = (att_unnorm @ feat) / rowsum(att_unnorm); att transposed
on PE, rowsum from a ones-column appended to the feat blocks; the reciprocal
scale rides the ACT copy's free per-partition scale operand.
Both i-tiles' pair loops are emitted before either softmax phase so the
scheduler overlaps i-tile 0's softmax with i-tile 1's pair loop.
"""
import sys

sys.path.insert(0, "/opt/trn_rl_repo")

from contextlib import ExitStack

import numpy as np

import concourse.bass as bass  # noqa: F401
import concourse.tile as tile
from concourse import bacc, masks, mybir
from concourse.bass_utils import run_bass_kernel_spmd

B, T, C_IN, D = 2, 1024, 128, 64
N_CORES = 8
ROWS = (B * T) // N_CORES  # 256 destination rows per core
CPB = N_CORES // B  # cores per batch
NT = T // 128  # token tiles
NIT = ROWS // 128  # i-tiles per core
NPAIR = 64  # row pairs per i-tile
NSLOT = 16  # pair slots per 32-row psum band

FP32 = mybir.dt.float32
FP16 = mybir.dt.float16
AX = mybir.AxisListType.X
OP = mybir.AluOpType
AF = mybir.ActivationFunctionType

MASK_BIAS = 30.0  # exp(s + 30*adj - 30): e^-30 zeroes masked entries in fp16


def _emit(ctx, tc, nc, featT16, featkT16, feat16b, wT16_in, adj, a32, out):
    singles = ctx.enter_context(tc.tile_pool(name="singles", bufs=1))
    ident16 = singles.tile([128, 128], FP16)
    masks.make_identity(nc, ident16[:])
    ident30 = singles.tile([128, 128], FP16)
    nc.vector.tensor_scalar(ident30[:], ident16[:], MASK_BIAS, None, OP.mult)
    nbias = singles.tile([128, 1], FP32)
    nc.vector.memset(nbias[:], -MASK_BIAS)
    feat16 = singles.tile([128, NT * (C_IN + 1)], FP16)  # feat blocks + ones col
    qT2 = singles.tile([128, T], FP16)
    kpair = singles.tile([128, ROWS // 2], FP32)
    A32s = singles.tile([128, NSLOT * 32], FP16)
    wT16 = singles.tile([128, 2 * D], FP16)
    adj_sb = singles.tile([128, NIT * T], FP16)  # both i-tiles' adj rows

    with ExitStack() as sctx:
        spsum = sctx.enter_context(tc.tile_pool(name="setup_ps", bufs=4, space="PSUM"))
        spool = sctx.enter_context(tc.tile_pool(name="setup_sb", bufs=1))

        # Spread input DMAs over the three issuing queues (sync/scalar/gpsimd
        # use separate DMA rings, and same-queue transfers serialize); each
        # DMA pays ~2.3us fixed latency (issue+DGE+sem-prop), so the three
        # tensors on the qT2/kpair critical path each lead their queue.
        fT = spool.tile([128, T], FP16, tag="fT")
        nc.sync.dma_start(fT[:], featT16[:, :])
        nc.sync.dma_start(wT16[:], wT16_in[:, :])
        fkT = spool.tile([128, ROWS], FP16, tag="fkT")
        nc.scalar.dma_start(fkT[:], featkT16[:, :])
        for it in range(NIT):
            nc.gpsimd.dma_start(
                adj_sb[:, it * T : (it + 1) * T], adj[it * 128 : (it + 1) * 128, :]
            )
        nc.gpsimd.dma_start(A32s[:], a32[:, :])
        nc.scalar.dma_start(feat16[:], feat16b[:, :])

        # kT = W2^T.T @ featkT  [64, ROWS] -> kpair columns [k(2p); k(2p+1)]
        kps = spsum.tile([64, ROWS], FP32, tag="qk")
        nc.tensor.matmul(kps[:], wT16[:, D : 2 * D], fkT[:], start=True, stop=True)
        kpv = kps[:].rearrange("d (p two) -> d two p", two=2)
        nc.vector.tensor_copy(kpair[0:64, :], kpv[:, 0, :])
        nc.vector.tensor_copy(kpair[64:128, :], kpv[:, 1, :])

        # qT = W1^T.T @ featT   [64, T] -> stacked fp16 qT2
        for h in range(T // 512):
            ps = spsum.tile([64, 512], FP32, tag="qk")
            nc.tensor.matmul(
                ps[:], wT16[:, 0:D], fT[:, h * 512 : (h + 1) * 512], start=True, stop=True
            )
            nc.vector.tensor_copy(qT2[0:64, h * 512 : (h + 1) * 512], ps[:])
            nc.scalar.copy(qT2[64:128, h * 512 : (h + 1) * 512], ps[:])

    # separate pools per producer so DVE buffer recycling never waits on the
    # slower ACT tiles' matmuls (idx%4==3 goes to ACT)
    e2pool = ctx.enter_context(tc.tile_pool(name="e2", bufs=5))
    e2vpool = ctx.enter_context(tc.tile_pool(name="e2v", bufs=4))
    e2apool = ctx.enter_context(tc.tile_pool(name="e2a", bufs=4))
    softpool = ctx.enter_context(tc.tile_pool(name="soft", bufs=2))
    smallpool = ctx.enter_context(tc.tile_pool(name="small", bufs=2))
    attTpool = ctx.enter_context(tc.tile_pool(name="attT", bufs=2))
    outpool = ctx.enter_context(tc.tile_pool(name="outp", bufs=2))
    ps_scores = ctx.enter_context(tc.tile_pool(name="ps_s", bufs=4, space="PSUM"))
    ps_tr = ctx.enter_context(tc.tile_pool(name="ps_tr", bufs=2, space="PSUM"))
    ps_out = ctx.enter_context(tc.tile_pool(name="ps_o", bufs=2, space="PSUM"))

    def emit_pairs(it, sc, lo_idx, hi_idx):
        s0, s1 = sc
        if lo_idx == 0:
            # open every band's accumulation group with the mask term
            # (s[band g] = 30 * adj[band g rows]); depends only on the adj
            # DMA, so all bands' openers run early and no mask work ever
            # trails the last score matmul.
            for g in range(4):
                i30 = ident30[32 * g : 32 * g + 32, 32 * g : 32 * g + 32]
                arows = adj_sb[32 * g : 32 * g + 32, it * T : (it + 1) * T]
                for hh, sps in enumerate((s0, s1)):
                    nc.tensor.matmul(
                        sps[32 * g : 32 * g + 32, :],
                        i30,
                        arows[:, hh * 512 : (hh + 1) * 512],
                        start=True,
                        stop=False,
                        tile_position=(32 * g, 32 * g),
                        skip_group_check=True,
                    )
        # visit pairs q-major so consecutive matmuls hit different PSUM
        # col-groups (tile_position col 32g) and overlap on the PE sub-arrays
        e2big = None
        for idx in range(lo_idx, hi_idx):
            q, g = divmod(idx, 4)
            p = NSLOT * g + q
            P = it * NPAIR + p
            r = idx % 4
            if r == 0:
                e2big = e2pool.tile([128, 2 * T], FP16, tag="e2")
                e2 = e2big[:, 0:T]
            elif r == 1:
                e2 = e2big[:, T : 2 * T]
            elif r == 2:
                e2v = e2vpool.tile([128, T], FP16, tag="e2v")
                e2 = e2v[:]
            else:
                e2a = e2apool.tile([128, T], FP16, tag="e2a")
                e2 = e2a[:]
            kcol = kpair[:, P : P + 1]
            if r == 3:
                nc.scalar.activation(e2[:], qT2[:], AF.Relu, bias=kcol)
            else:
                nc.vector.tensor_scalar(e2[:], qT2[:], kcol, 0.0, OP.add, OP.max)
            lhsT = A32s[:, 32 * q : 32 * q + 32]
            last = q == NSLOT - 1
            nc.tensor.matmul(
                s0[32 * g : 32 * g + 32, :],
                lhsT,
                e2[:, 0:512],
                start=False,
                stop=last,
                tile_position=(0, 32 * g),
                skip_group_check=True,
            )
            nc.tensor.matmul(
                s1[32 * g : 32 * g + 32, :],
                lhsT,
                e2[:, 512:T],
                start=False,
                stop=last,
                tile_position=(0, 32 * g),
                skip_group_check=True,
            )

    def emit_softmax_out(it, sc):
        s0, s1 = sc
        pexp = softpool.tile([128, T], FP16, tag="pexp")
        pst = ps_tr.tile([128, T], FP16, tag="tr")
        attT = attTpool.tile([128, T], FP16, tag="attT")
        for hh in range(2):
            lo = hh * 512
            nc.scalar.activation(
                pexp[:, lo : lo + 512], (s0, s1)[hh][:], AF.Exp, bias=nbias[:]
            )
            for t in range(lo // 128, (lo + 512) // 128):
                nc.tensor.transpose(
                    pst[:, t * 128 : (t + 1) * 128], pexp[:, t * 128 : (t + 1) * 128], ident16[:]
                )
            # PSUM->SBUF copies split across the two capable engines
            if hh == 0:
                nc.scalar.copy(attT[:, lo : lo + 512], pst[:, lo : lo + 512])
            else:
                nc.vector.tensor_copy(attT[:, lo : lo + 512], pst[:, lo : lo + 512])

        W = C_IN + 1
        po = ps_out.tile([128, W], FP32, tag="o")
        for t in range(NT):
            nc.tensor.matmul(
                po[:],
                attT[:, t * 128 : (t + 1) * 128],
                feat16[:, t * W : (t + 1) * W],
                start=(t == 0),
                stop=(t == NT - 1),
            )
        inv = smallpool.tile([128, 1], FP32, tag="inv")
        nc.vector.reciprocal(inv[:], po[:, C_IN : C_IN + 1])
        out_sb = outpool.tile([128, C_IN], FP32, tag="out")
        nc.scalar.activation(out_sb[:], po[:, 0:C_IN], AF.Copy, bias=0.0, scale=inv[:])
        nc.sync.dma_start(out[it * 128 : (it + 1) * 128, :], out_sb[:])

    # Emission order: i-tile 0's softmax is injected a few pairs into i-tile
    # 1's loop so the in-order Vector/Scalar queues reach it right when its
    # scores close, without stalling either queue.
    s0a = ps_scores.tile([128, 512], FP32, tag="s")
    s0b = ps_scores.tile([128, 512], FP32, tag="s")
    s1a = ps_scores.tile([128, 512], FP32, tag="s")
    s1b = ps_scores.tile([128, 512], FP32, tag="s")
    sc0 = (s0a, s0b)
    sc1 = (s1a, s1b)
    emit_pairs(0, sc0, 0, NPAIR)
    emit_pairs(1, sc1, 0, 16)
    emit_softmax_out(0, sc0)
    emit_pairs(1, sc1, 16, NPAIR)
    emit_softmax_out(1, sc1)


_PROGRAM = None


def build_program():
    global _PROGRAM
    if _PROGRAM is not None:
        return _PROGRAM
    nc = bacc.Bacc("TRN2", target_bir_lowering=False, debug=False, num_devices=N_CORES)
    featT16 = nc.dram_tensor("featT16", [C_IN, T], FP16, kind="ExternalInput")
    featkT16 = nc.dram_tensor("featkT16", [C_IN, ROWS], FP16, kind="ExternalInput")
    feat16b = nc.dram_tensor("feat16b", [128, NT * (C_IN + 1)], FP16, kind="ExternalInput")
    wT16_in = nc.dram_tensor("wT16", [C_IN, 2 * D], FP16, kind="ExternalInput")
    adj = nc.dram_tensor("adj", [ROWS, T], FP16, kind="ExternalInput")
    a32 = nc.dram_tensor("a32", [128, NSLOT * 32], FP16, kind="ExternalInput")
    out = nc.dram_tensor("out", [ROWS, C_IN], FP32, kind="ExternalOutput")
    with tile.TileContext(nc) as tc:
        with ExitStack() as ctx:
            _emit(ctx, tc, nc, featT16, featkT16, feat16b, wT16_in, adj, a32, out)
    nc.compile()
    _PROGRAM = nc
    return nc


def make_a32(a):
    a32 = np.zeros((128, NSLOT * 32), dtype=np.float16)
    for q in range(NSLOT):
        a32[0:64, 32 * q + 2 * q] = a
        a32[64:128, 32 * q + 2 * q + 1] = a
    return a32


def make_in_maps(feat, adj, W1, W2, a):
    feat = np.ascontiguousarray(feat, dtype=np.float32)
    adj = np.ascontiguousarray(adj, dtype=np.float32)
    W1 = np.asarray(W1, dtype=np.float32)
    W2 = np.asarray(W2, dtype=np.float32)
    a32 = make_a32(np.asarray(a, dtype=np.float32))
    wT16 = np.ascontiguousarray(
        np.concatenate([W1.T, W2.T], axis=1).astype(np.float16)
    )  # [128, 128]
    in_maps = []
    for b in range(B):
        feat16 = feat[b].astype(np.float16)  # [T, C_IN]
        fT = np.ascontiguousarray(feat16.T)  # [C_IN, T]
        fb = feat16.reshape(NT, 128, C_IN).transpose(1, 0, 2)  # [128, NT, C_IN]
        fblk = np.concatenate(
            [fb, np.ones((128, NT, 1), dtype=np.float16)], axis=2
        ).reshape(128, NT * (C_IN + 1))
        fblk = np.ascontiguousarray(fblk)
        for cc in range(CPB):
            r0 = cc * ROWS
            in_maps.append(
                {
                    "featT16": fT,
                    "featkT16": np.ascontiguousarray(fT[:, r0 : r0 + ROWS]),
                    "feat16b": fblk,
                    "wT16": wT16,
                    "adj": np.ascontiguousarray(adj[b, r0 : r0 + ROWS].astype(np.float16)),
                    "a32": a32,
                }
            )
    return in_maps


def run(feat, adj, W1, W2, a, trace=False):
    nc = build_program()
    in_maps = make_in_maps(feat, adj, W1, W2, a)
    last_err = None
    for attempt in range(3):
        try:
            res = run_bass_kernel_spmd(
                nc, in_maps, core_ids=list(range(N_CORES)), trace=trace
            )
            outs = [np.asarray(res.results[c]["out"]) for c in range(N_CORES)]
            break
        except Exception as e:  # transient NRT device errors recover on retry
            last_err = e
            import time

            time.sleep(5)
    else:
        raise last_err
    full = np.concatenate(outs, axis=0).reshape(B, T, C_IN).astype(np.float32)
    return full, res


def kernel(feat, adj, W1, W2, a):
    full, _ = run(feat, adj, W1, W2, a)
    return full


# revision 19
# speedup vs baseline: 1.0870x; 1.0141x over previous
"""GATv2 attention layer (B=2, T=1024, C_IN=128, D=64) on 8 trn2 NeuronCores.

Sharding: flatten (B, T) destination rows -> 2048 rows, 256 per core.
Each core gets fp16 host-prepared layouts: feat^T of its batch, its own 256
rows' feat^T slice (for k), feat in 128-row blocks (final matmul rhs),
[W1^T | W2^T], the score weight A32s, plus its fp16 adj rows.

Per-core algorithm (i = destination row, j = source node, d = head dim 64):
  scores[i, j] = sum_d a[d] * relu(q[j, d] + k[i, d])
Layout trick: qT2 = [q^T; q^T] stacked [128(=2x64 d), 1024(=j)] in fp16.
For a PAIR of rows (2p, 2p+1), bias column kpair[:, p] = [k[2p]; k[2p+1]]:
  E2 = relu(qT2 + kpair[:, p])   one DVE tensor_scalar / ACT activation
  (DVE at 4 fp16/cyc/lane takes 48 of the 64 pair tiles, ACT 1/cyc takes 16;
  this 3:1 split matches their measured rates, both engines saturate)
  scores come from a PE matmul with lhsT = A32s slot q=p%16, an [128, 32]
  fp16 matrix holding `a` in column 2q (top d-half) and 2q+1 (bottom d-half),
  zeros elsewhere. 16 pairs accumulate into one 32-row psum band, so the
  matmul psum base stays 32-aligned (hardware requirement) while every
  logical row ends up at psum partition 2p+{0,1}. Consecutive matmuls are
  issued to different PSUM col-groups so they overlap on the PE sub-arrays.
Mask fold: softmax is shift-invariant, so exp(s)*adj == exp(s + 30*adj - 30)
(e^-30 ~ 1e-13 kills masked entries; unmasked are exact). The +30*adj term
is accumulated INTO the score psum by one extra small matmul per 32-row band
(lhsT = a 30-scaled identity block, rhs = this band's adj rows), and the -30
rides the exp activation's free bias — the mask costs the Vector engine
nothing.
Final: out[i, :] = (att_unnorm @ feat) / rowsum(att_unnorm); att transposed
on PE, rowsum from a ones-column appended to the feat blocks; the reciprocal
scale rides the ACT copy's free per-partition scale operand.
Both i-tiles' pair loops are emitted before either softmax phase so the
scheduler overlaps i-tile 0's softmax with i-tile 1's pair loop.
"""
import sys

sys.path.insert(0, "/opt/trn_rl_repo")

from contextlib import ExitStack

import numpy as np

import concourse.bass as bass  # noqa: F401
import concourse.tile as tile
from concourse import bacc, masks, mybir
from concourse.bass_utils import run_bass_kernel_spmd

B, T, C_IN, D = 2, 1024, 128, 64
N_CORES = 8
ROWS = (B * T) // N_CORES  # 256 destination rows per core
CPB = N_CORES // B  # cores per batch
NT = T // 128  # token tiles
NIT = ROWS // 128  # i-tiles per core
NPAIR = 64  # row pairs per i-tile
NSLOT = 16  # pair slots per 32-row psum band

FP32 = mybir.dt.float32
FP16 = mybir.dt.float16
AX = mybir.AxisListType.X
OP = mybir.AluOpType
AF = mybir.ActivationFunctionType

MASK_BIAS = 30.0  # exp(s + 30*adj - 30): e^-30 zeroes masked entries in fp16


def _emit(ctx, tc, nc, featT16, featkT16, feat16b, wT16_in, adj, a32, out):
    singles = ctx.enter_context(tc.tile_pool(name="singles", bufs=1))
    ident16 = singles.tile([128, 128], FP16)
    masks.make_identity(nc, ident16[:])
    ident30 = singles.tile([128, 128], FP16)
    nc.vector.tensor_scalar(ident30[:], ident16[:], MASK_BIAS, None, OP.mult)
    nbias = singles.tile([128, 1], FP32)
    nc.vector.memset(nbias[:], -MASK_BIAS)
    feat16 = singles.tile([128, NT * (C_IN + 1)], FP16)  # feat blocks + ones col
    qT2 = singles.tile([128, T], FP16)
    kpair = singles.tile([128, ROWS // 2], FP32)
    A32s = singles.tile([128, NSLOT * 32], FP16)
    wT16 = singles.tile([128, 2 * D], FP16)
    adj_sb = singles.tile([128, NIT * T], FP16)  # both i-tiles' adj rows

    with ExitStack() as sctx:
        spsum = sctx.enter_context(tc.tile_pool(name="setup_ps", bufs=4, space="PSUM"))
        spool = sctx.enter_context(tc.tile_pool(name="setup_sb", bufs=1))

        # Spread input DMAs over the three issuing queues (sync/scalar/gpsimd
        # use separate DMA rings, and same-queue transfers serialize); each
        # DMA pays ~2.3us fixed latency (issue+DGE+sem-prop), so the three
        # tensors on the qT2/kpair critical path each lead their queue.
        nc.sync.dma_start(wT16[:], wT16_in[:, :])
        fT = spool.tile([128, T], FP16, tag="fT")
        nc.sync.dma_start(fT[:], featT16[:, :])
        fkT = spool.tile([128, ROWS], FP16, tag="fkT")
        nc.scalar.dma_start(fkT[:], featkT16[:, :])
        for it in range(NIT):
            nc.gpsimd.dma_start(
                adj_sb[:, it * T : (it + 1) * T], adj[it * 128 : (it + 1) * 128, :]
            )
        nc.gpsimd.dma_start(A32s[:], a32[:, :])
        nc.scalar.dma_start(feat16[:], feat16b[:, :])

        # kT = W2^T.T @ featkT  [64, ROWS] -> kpair columns [k(2p); k(2p+1)]
        kps = spsum.tile([64, ROWS], FP32, tag="qk")
        nc.tensor.matmul(kps[:], wT16[:, D : 2 * D], fkT[:], start=True, stop=True)
        kpv = kps[:].rearrange("d (p two) -> d two p", two=2)
        nc.vector.tensor_copy(kpair[0:64, :], kpv[:, 0, :])
        nc.vector.tensor_copy(kpair[64:128, :], kpv[:, 1, :])

        # qT = W1^T.T @ featT   [64, T] -> stacked fp16 qT2
        for h in range(T // 512):
            ps = spsum.tile([64, 512], FP32, tag="qk")
            nc.tensor.matmul(
                ps[:], wT16[:, 0:D], fT[:, h * 512 : (h + 1) * 512], start=True, stop=True
            )
            nc.vector.tensor_copy(qT2[0:64, h * 512 : (h + 1) * 512], ps[:])
            nc.scalar.copy(qT2[64:128, h * 512 : (h + 1) * 512], ps[:])

    # separate pools per producer so DVE buffer recycling never waits on the
    # slower ACT tiles' matmuls (idx%4==3 goes to ACT)
    e2pool = ctx.enter_context(tc.tile_pool(name="e2", bufs=5))
    e2vpool = ctx.enter_context(tc.tile_pool(name="e2v", bufs=4))
    e2apool = ctx.enter_context(tc.tile_pool(name="e2a", bufs=4))
    softpool = ctx.enter_context(tc.tile_pool(name="soft", bufs=2))
    smallpool = ctx.enter_context(tc.tile_pool(name="small", bufs=2))
    attTpool = ctx.enter_context(tc.tile_pool(name="attT", bufs=2))
    outpool = ctx.enter_context(tc.tile_pool(name="outp", bufs=2))
    ps_scores = ctx.enter_context(tc.tile_pool(name="ps_s", bufs=4, space="PSUM"))
    ps_tr = ctx.enter_context(tc.tile_pool(name="ps_tr", bufs=2, space="PSUM"))
    ps_out = ctx.enter_context(tc.tile_pool(name="ps_o", bufs=2, space="PSUM"))

    def emit_pairs(it, sc, lo_idx, hi_idx):
        s0, s1 = sc
        if lo_idx == 0:
            # open every band's accumulation group with the mask term
            # (s[band g] = 30 * adj[band g rows]); depends only on the adj
            # DMA, so all bands' openers run early and no mask work ever
            # trails the last score matmul.
            for g in range(4):
                i30 = ident30[32 * g : 32 * g + 32, 32 * g : 32 * g + 32]
                arows = adj_sb[32 * g : 32 * g + 32, it * T : (it + 1) * T]
                for hh, sps in enumerate((s0, s1)):
                    nc.tensor.matmul(
                        sps[32 * g : 32 * g + 32, :],
                        i30,
                        arows[:, hh * 512 : (hh + 1) * 512],
                        start=True,
                        stop=False,
                        tile_position=(32 * g, 32 * g),
                        skip_group_check=True,
                    )
        # visit pairs q-major so consecutive matmuls hit different PSUM
        # col-groups (tile_position col 32g) and overlap on the PE sub-arrays
        e2big = None
        for idx in range(lo_idx, hi_idx):
            q, g = divmod(idx, 4)
            p = NSLOT * g + q
            P = it * NPAIR + p
            r = idx % 4
            if r == 0:
                e2big = e2pool.tile([128, 2 * T], FP16, tag="e2")
                e2 = e2big[:, 0:T]
            elif r == 1:
                e2 = e2big[:, T : 2 * T]
            elif r == 2:
                e2v = e2vpool.tile([128, T], FP16, tag="e2v")
                e2 = e2v[:]
            else:
                e2a = e2apool.tile([128, T], FP16, tag="e2a")
                e2 = e2a[:]
            kcol = kpair[:, P : P + 1]
            # last i-tile's final ACT slot goes to Vector instead, so the
            # Scalar queue is free to start the tail exp immediately
            if r == 3 and not (it == NIT - 1 and idx >= 56):
                nc.scalar.activation(e2[:], qT2[:], AF.Relu, bias=kcol)
            else:
                nc.vector.tensor_scalar(e2[:], qT2[:], kcol, 0.0, OP.add, OP.max)
            lhsT = A32s[:, 32 * q : 32 * q + 32]
            last = q == NSLOT - 1
            nc.tensor.matmul(
                s0[32 * g : 32 * g + 32, :],
                lhsT,
                e2[:, 0:512],
                start=False,
                stop=last,
                tile_position=(0, 32 * g),
                skip_group_check=True,
            )
            nc.tensor.matmul(
                s1[32 * g : 32 * g + 32, :],
                lhsT,
                e2[:, 512:T],
                start=False,
                stop=last,
                tile_position=(0, 32 * g),
                skip_group_check=True,
            )

    def emit_softmax(it, sc, split_out=False):
        s0, s1 = sc
        pexp = softpool.tile([128, T], FP16, tag="pexp")
        pst = ps_tr.tile([128, T], FP16, tag="tr")
        attT = attTpool.tile([128, T], FP16, tag="attT")
        W = C_IN + 1
        if split_out:
            po = ps_out.tile([128, W], FP32, tag="o")
        else:
            po = None
        for hh in range(2):
            lo = hh * 512
            nc.scalar.activation(
                pexp[:, lo : lo + 512], (s0, s1)[hh][:], AF.Exp, bias=nbias[:]
            )
            for t in range(lo // 128, (lo + 512) // 128):
                nc.tensor.transpose(
                    pst[:, t * 128 : (t + 1) * 128], pexp[:, t * 128 : (t + 1) * 128], ident16[:]
                )
            # PSUM->SBUF copies split across the two capable engines
            if hh == 0:
                nc.scalar.copy(attT[:, lo : lo + 512], pst[:, lo : lo + 512])
            else:
                nc.vector.tensor_copy(attT[:, lo : lo + 512], pst[:, lo : lo + 512])
            if split_out:
                # last i-tile: accumulate the output matmul per j-half so the
                # tail chain after the final copy is as short as possible
                for t in range(lo // 128, (lo + 512) // 128):
                    nc.tensor.matmul(
                        po[:],
                        attT[:, t * 128 : (t + 1) * 128],
                        feat16[:, t * W : (t + 1) * W],
                        start=(t == 0),
                        stop=(t == NT - 1),
                    )
        return attT, po

    def emit_out(it, attT, po):
        W = C_IN + 1
        if po is None:
            po = ps_out.tile([128, W], FP32, tag="o")
            for t in range(NT):
                nc.tensor.matmul(
                    po[:],
                    attT[:, t * 128 : (t + 1) * 128],
                    feat16[:, t * W : (t + 1) * W],
                    start=(t == 0),
                    stop=(t == NT - 1),
                )
        inv = smallpool.tile([128, 1], FP32, tag="inv")
        nc.vector.reciprocal(inv[:], po[:, C_IN : C_IN + 1])
        out_sb = outpool.tile([128, C_IN], FP32, tag="out")
        nc.scalar.activation(out_sb[:], po[:, 0:C_IN], AF.Copy, bias=0.0, scale=inv[:])
        nc.sync.dma_start(out[it * 128 : (it + 1) * 128, :], out_sb[:])

    # Emission order: i-tile 0's softmax and output stages are injected at
    # staggered points of i-tile 1's pair loop so the in-order Vector/Scalar
    # queues reach each op just when its inputs are ready, never stalling.
    s0a = ps_scores.tile([128, 512], FP32, tag="s")
    s0b = ps_scores.tile([128, 512], FP32, tag="s")
    s1a = ps_scores.tile([128, 512], FP32, tag="s")
    s1b = ps_scores.tile([128, 512], FP32, tag="s")
    sc0 = (s0a, s0b)
    sc1 = (s1a, s1b)
    emit_pairs(0, sc0, 0, NPAIR)
    emit_pairs(1, sc1, 0, 16)
    attT0, po0 = emit_softmax(0, sc0)
    emit_pairs(1, sc1, 16, 32)
    emit_out(0, attT0, po0)
    emit_pairs(1, sc1, 32, NPAIR)
    attT1, po1 = emit_softmax(1, sc1, split_out=True)
    emit_out(1, attT1, po1)


_PROGRAM = None


def build_program():
    global _PROGRAM
    if _PROGRAM is not None:
        return _PROGRAM
    nc = bacc.Bacc("TRN2", target_bir_lowering=False, debug=False, num_devices=N_CORES)
    featT16 = nc.dram_tensor("featT16", [C_IN, T], FP16, kind="ExternalInput")
    featkT16 = nc.dram_tensor("featkT16", [C_IN, ROWS], FP16, kind="ExternalInput")
    feat16b = nc.dram_tensor("feat16b", [128, NT * (C_IN + 1)], FP16, kind="ExternalInput")
    wT16_in = nc.dram_tensor("wT16", [C_IN, 2 * D], FP16, kind="ExternalInput")
    adj = nc.dram_tensor("adj", [ROWS, T], FP16, kind="ExternalInput")
    a32 = nc.dram_tensor("a32", [128, NSLOT * 32], FP16, kind="ExternalInput")
    out = nc.dram_tensor("out", [ROWS, C_IN], FP32, kind="ExternalOutput")
    with tile.TileContext(nc) as tc:
        with ExitStack() as ctx:
            _emit(ctx, tc, nc, featT16, featkT16, feat16b, wT16_in, adj, a32, out)
    nc.compile()
    _PROGRAM = nc
    return nc


def make_a32(a):
    a32 = np.zeros((128, NSLOT * 32), dtype=np.float16)
    for q in range(NSLOT):
        a32[0:64, 32 * q + 2 * q] = a
        a32[64:128, 32 * q + 2 * q + 1] = a
    return a32


def make_in_maps(feat, adj, W1, W2, a):
    feat = np.ascontiguousarray(feat, dtype=np.float32)
    adj = np.ascontiguousarray(adj, dtype=np.float32)
    W1 = np.asarray(W1, dtype=np.float32)
    W2 = np.asarray(W2, dtype=np.float32)
    a32 = make_a32(np.asarray(a, dtype=np.float32))
    wT16 = np.ascontiguousarray(
        np.concatenate([W1.T, W2.T], axis=1).astype(np.float16)
    )  # [128, 128]
    in_maps = []
    for b in range(B):
        feat16 = feat[b].astype(np.float16)  # [T, C_IN]
        fT = np.ascontiguousarray(feat16.T)  # [C_IN, T]
        fb = feat16.reshape(NT, 128, C_IN).transpose(1, 0, 2)  # [128, NT, C_IN]
        fblk = np.concatenate(
            [fb, np.ones((128, NT, 1), dtype=np.float16)], axis=2
        ).reshape(128, NT * (C_IN + 1))
        fblk = np.ascontiguousarray(fblk)
        for cc in range(CPB):
            r0 = cc * ROWS
            in_maps.append(
                {
                    "featT16": fT,
                    "featkT16": np.ascontiguousarray(fT[:, r0 : r0 + ROWS]),
                    "feat16b": fblk,
                    "wT16": wT16,
                    "adj": np.ascontiguousarray(adj[b, r0 : r0 + ROWS].astype(np.float16)),
                    "a32": a32,
                }
            )
    return in_maps


def run(feat, adj, W1, W2, a, trace=False):
    nc = build_program()
    in_maps = make_in_maps(feat, adj, W1, W2, a)
    last_err = None
    for attempt in range(3):
        try:
            res = run_bass_kernel_spmd(
                nc, in_maps, core_ids=list(range(N_CORES)), trace=trace
            )
            outs = [np.asarray(res.results[c]["out"]) for c in range(N_CORES)]
            break
        except Exception as e:  # transient NRT device errors recover on retry
            last_err = e
            import time

            time.sleep(5)
    else:
        raise last_err
    full = np.concatenate(outs, axis=0).reshape(B, T, C_IN).astype(np.float32)
    return full, res


def kernel(feat, adj, W1, W2, a):
    full, _ = run(feat, adj, W1, W2, a)
    return full
